# revision 21
# baseline (speedup 1.0000x reference)
"""BiMamba block (bidirectional Mamba-1 + layernorm) as a Bass/Tile kernel
for 8 Trainium2 NeuronCores.

Sharding: data-parallel over batch — core i computes batch row i end-to-end
(both scan directions + layernorm), no collectives.

Per-core layout: channel-major [channel(partition), time(free)] activations
until the output projection, which emits token-major [token, d_model].

Selective scan per (d-block of 128 channels, state index n) on [128, L] tiles:
    a = exp(A[:,n] * dt)            ACT, per-partition scale AP
    b = (dt*u) * B_bcast[n]         DVE tensor_tensor bf16 (2x mode)
    h = tensor_tensor_scan(a, b)    DVE, fp32 internal recurrence state
    q = h * C_bcast[n]              DVE tensor_tensor bf16 (2x mode)
    psum_y += I @ q                 PE accumulates the sum over n in PSUM

B/C broadcasts are DMA re-reads of a small DRAM staging row with a
partition-step-0 access pattern.  The backward direction runs on
host-reversed input; un-reversal is free via a negative-stride output AP at
the yf write.
"""

import os
import sys
from contextlib import ExitStack

for _p in ("/opt/trn_rl_repo", "/root/.axon_site/_ro/trn_rl_repo"):
    if os.path.isdir(_p) and _p not in sys.path:
        sys.path.insert(0, _p)

import numpy as np
import ml_dtypes

import concourse.bass as bass
import concourse.tile as tile
from concourse import bacc, mybir
from concourse.masks import make_identity

AF = mybir.ActivationFunctionType
ALU = mybir.AluOpType
F32 = mybir.dt.float32
F32R = mybir.dt.float32r
BF16 = mybir.dt.bfloat16
F16 = mybir.dt.float16

D_MODEL = 512
D_STATE = 16
D_CONV = 4
D_INNER = 1024
DT_RANK = 32
NB = D_INNER // 128          # 8 d-blocks
KM = D_MODEL // 128          # 4 k-tiles over d_model
LN_EPS = 1e-5

XZ_F32R = False              # xz matmul in float32r (else bf16)
SCAN_B_BF16 = True           # scan data1 dtype bf16 (else fp32)
STOP_AFTER = os.environ.get("BIMAMBA_STOP_AFTER", "")


def host_prep(inputs: dict, l_override: int | None = None) -> tuple[list[dict], int]:
    """Full problem inputs -> per-core in_maps (one batch row per core)."""
    x = np.asarray(inputs["x"], dtype=np.float32)
    Bsz, L, _ = x.shape
    if l_override is not None:
        L = l_override
        x = x[:, :L]
    bf = ml_dtypes.bfloat16

    def pack(a, nblk):  # [nblk*128, F] -> [128, nblk*F]
        return np.concatenate([a[i * 128:(i + 1) * 128] for i in range(nblk)], axis=1).copy()

    shared = {}
    for p in ("f", "b"):
        Wxz = np.asarray(inputs[f"{p}_Wxz"], np.float32)
        shared[f"{p}_Wxz"] = pack(Wxz, KM) if XZ_F32R else pack(Wxz, KM).astype(bf)
        cw = np.asarray(inputs[f"{p}_conv_w"], np.float32).reshape(D_INNER, D_CONV)
        shared[f"{p}_convw"] = pack(cw, NB)
        cb = np.asarray(inputs[f"{p}_conv_b"], np.float32).reshape(D_INNER, 1)
        shared[f"{p}_convb"] = pack(cb, NB)
        Wxm = np.asarray(inputs[f"{p}_Wx"], np.float32)
        Wxp = np.zeros((D_INNER, 80), np.float32)
        Wxp[:, 0:48] = Wxm[:, 0:48]
        Wxp[:, 64:80] = Wxm[:, 48:64]
        shared[f"{p}_Wx"] = pack(Wxp, NB).astype(bf)
        Wdtf = np.asarray(inputs[f"{p}_Wdt"], np.float32)
        Wdt_hi = Wdtf.astype(bf)
        Wdt_lo = (Wdtf - Wdt_hi.astype(np.float32)).astype(bf)
        shared[f"{p}_Wdt"] = np.concatenate([Wdt_hi, Wdt_lo], axis=1).copy()
        nbdt = -np.asarray(inputs[f"{p}_bdt"], np.float32).reshape(D_INNER, 1)
        shared[f"{p}_bdt"] = pack(nbdt, NB)
        # negated-dt convention: dtn = -softplus(...) is stored, so the exp
        # scale must be -A = +exp(A_log)
        negA = np.exp(np.asarray(inputs[f"{p}_A_log"], np.float32))
        shared[f"{p}_A"] = pack(negA, NB)
        Dv = np.asarray(inputs[f"{p}_D"], np.float32).reshape(D_INNER, 1)
        shared[f"{p}_D"] = pack(Dv, NB)
        shared[f"{p}_Wout"] = pack(np.asarray(inputs[f"{p}_Wout"], np.float32), NB).astype(bf)
    shared["ln_g"] = np.broadcast_to(np.asarray(inputs["ln_g"], np.float32)[None, :], (128, D_MODEL)).copy()
    shared["ln_b"] = np.broadcast_to(np.asarray(inputs["ln_b"], np.float32)[None, :], (128, D_MODEL)).copy()

    in_maps = []
    for bi in range(Bsz):
        xT = np.ascontiguousarray(x[bi].T)        # [512, L]
        xTr = np.ascontiguousarray(x[bi][::-1].T)
        m = dict(shared)
        m["xT"] = pack(xT, KM) if XZ_F32R else pack(xT, KM).astype(bf)
        m["xTr"] = pack(xTr, KM) if XZ_F32R else pack(xTr, KM).astype(bf)
        in_maps.append(m)
    return in_maps, L


def declare_ios(nc: bass.Bass, L: int) -> dict:
    io = {}
    xdt = F32R if XZ_F32R else BF16
    io["xT"] = nc.dram_tensor("xT", [128, KM * L], xdt, kind="ExternalInput").ap()
    io["xTr"] = nc.dram_tensor("xTr", [128, KM * L], xdt, kind="ExternalInput").ap()
    for p in ("f", "b"):
        io[f"{p}_Wxz"] = nc.dram_tensor(f"{p}_Wxz", [128, KM * 2 * D_INNER], xdt, kind="ExternalInput").ap()
        io[f"{p}_convw"] = nc.dram_tensor(f"{p}_convw", [128, NB * D_CONV], F32, kind="ExternalInput").ap()
        io[f"{p}_convb"] = nc.dram_tensor(f"{p}_convb", [128, NB], F32, kind="ExternalInput").ap()
        io[f"{p}_Wx"] = nc.dram_tensor(f"{p}_Wx", [128, NB * 80], BF16, kind="ExternalInput").ap()
        io[f"{p}_Wdt"] = nc.dram_tensor(f"{p}_Wdt", [32, 2 * D_INNER], BF16, kind="ExternalInput").ap()
        io[f"{p}_bdt"] = nc.dram_tensor(f"{p}_bdt", [128, NB], F32, kind="ExternalInput").ap()
        io[f"{p}_A"] = nc.dram_tensor(f"{p}_A", [128, NB * D_STATE], F32, kind="ExternalInput").ap()
        io[f"{p}_D"] = nc.dram_tensor(f"{p}_D", [128, NB], F32, kind="ExternalInput").ap()
        io[f"{p}_Wout"] = nc.dram_tensor(f"{p}_Wout", [128, NB * D_MODEL], BF16, kind="ExternalInput").ap()
    io["ln_g"] = nc.dram_tensor("ln_g", [128, D_MODEL], F32, kind="ExternalInput").ap()
    io["ln_b"] = nc.dram_tensor("ln_b", [128, D_MODEL], F32, kind="ExternalInput").ap()
    io["out"] = nc.dram_tensor("out", [L, D_MODEL], F32, kind="ExternalOutput").ap()
    return io


def build_kernel(ctx: ExitStack, tc: tile.TileContext, io: dict, L: int):
    nc = tc.nc
    FC = min(512, L)
    FT = L // FC                 # 512-wide free chunks
    MT = L // 128                # token tiles
    xdt = F32R if XZ_F32R else BF16
    bdtype = BF16 if SCAN_B_BF16 else F32
    HM = D_INNER // 128          # m-tiles per xz half (8)

    wpool = ctx.enter_context(tc.tile_pool(name="wglob", bufs=1))
    ident = wpool.tile([128, 128], BF16, tag="ident")
    make_identity(nc, ident[:])
    ln_g = wpool.tile([128, D_MODEL], F32, tag="ln_g")
    nc.sync.dma_start(ln_g[:], io["ln_g"])
    ln_b = wpool.tile([128, D_MODEL], F32, tag="ln_b")
    nc.sync.dma_start(ln_b[:], io["ln_b"])
    dglob = ctx.enter_context(tc.tile_pool(name="dglob", bufs=1, space="DRAM"))
    s_d = dglob.tile([128, MT * D_MODEL], F32, tag="s_d")

    for p in ("f", "b"):
        with ExitStack() as dctx:
            awpool = dctx.enter_context(tc.tile_pool(name=f"aw{p}", bufs=1))
            ucpool = dctx.enter_context(tc.tile_pool(name=f"ucp{p}", bufs=1))
            dpool = dctx.enter_context(tc.tile_pool(name=f"dram{p}", bufs=1, space="DRAM"))
            zs_d = dpool.tile([128, NB * L], BF16, tag="zs_d")
            bc_d = dpool.tile([32, L], BF16, tag="bc_d")

            wx = awpool.tile([128, NB * 80], BF16, tag="wx")
            nc.sync.dma_start(wx[:], io[f"{p}_Wx"])
            amat = awpool.tile([128, NB * D_STATE], F32, tag="amat")
            nc.sync.dma_start(amat[:], io[f"{p}_A"])
            dmat = awpool.tile([128, NB], F32, tag="dmat")
            nc.sync.dma_start(dmat[:], io[f"{p}_D"])
            wout = awpool.tile([128, NB * D_MODEL], BF16, tag="wout")
            nc.sync.dma_start(wout[:], io[f"{p}_Wout"])

            uc = [ucpool.tile([128, L], BF16, tag=f"uc{d}", name=f"uc{d}") for d in range(NB)]

            # ---------- phase A: xz matmul (m-half-streamed weights), conv ----------
            with ExitStack() as actx:
                apool = actx.enter_context(tc.tile_pool(name=f"pa{p}", bufs=1))
                whpool = actx.enter_context(tc.tile_pool(name=f"wh{p}", bufs=2))
                u0pool = actx.enter_context(tc.tile_pool(name=f"u0p{p}", bufs=3))
                evpool = actx.enter_context(tc.tile_pool(name=f"ev{p}", bufs=3))
                convpool = actx.enter_context(tc.tile_pool(name=f"conv{p}", bufs=2))
                psA = actx.enter_context(tc.tile_pool(name=f"psA{p}", bufs=3, space="PSUM"))

                xin = apool.tile([128, KM * L], xdt, tag="xin")
                nc.sync.dma_start(xin[:], io["xT" if p == "f" else "xTr"])
                convw = apool.tile([128, NB * D_CONV], F32, tag="convw")
                nc.sync.dma_start(convw[:], io[f"{p}_convw"])
                convb = apool.tile([128, NB], F32, tag="convb")
                nc.sync.dma_start(convb[:], io[f"{p}_convb"])

                for half in range(2):        # 0: u-channels, 1: z-channels
                    wh = whpool.tile([128, KM * D_INNER], xdt, tag="wh")
                    wsrc = io[f"{p}_Wxz"].rearrange("p (k c) -> p k c", k=KM)[
                        :, :, half * D_INNER:(half + 1) * D_INNER]
                    nc.sync.dma_start(
                        wh[:].rearrange("p (k c) -> p k c", k=KM), wsrc)
                    for m8 in range(HM):
                        u0t = None
                        if half == 0:
                            u0t = u0pool.tile([128, D_CONV - 1 + L], BF16, tag="u0")
                            nc.gpsimd.memset(u0t[:, 0:D_CONV - 1], 0.0)
                        for f in range(FT):
                            ps = psA.tile([128, FC], F32, tag="pxz")
                            for k in range(KM):
                                nc.tensor.matmul(
                                    ps[:],
                                    wh[:, k * D_INNER + m8 * 128: k * D_INNER + (m8 + 1) * 128],
                                    xin[:, k * L + f * FC: k * L + (f + 1) * FC],
                                    start=(k == 0), stop=(k == KM - 1),
                                )
                            if half == 0:
                                nc.scalar.copy(
                                    u0t[:, D_CONV - 1 + f * FC: D_CONV - 1 + (f + 1) * FC], ps[:])
                            else:
                                zt = evpool.tile([128, FC], BF16, tag="zt")
                                nc.scalar.activation(zt[:], ps[:], AF.Silu)
                                nc.sync.dma_start(
                                    zs_d[:, m8 * L + f * FC: m8 * L + (f + 1) * FC], zt[:])
                        if half == 0:
                            # depthwise causal conv + silu for d-block m8
                            for f in range(FT):
                                acc = [convpool.tile([128, FC], F32, tag=f"cacc{j % 2}",
                                                     name=f"cacc{j}") for j in range(D_CONV)]
                                nc.vector.tensor_scalar_mul(
                                    acc[0][:], u0t[:, f * FC: f * FC + FC],
                                    convw[:, m8 * D_CONV: m8 * D_CONV + 1])
                                for j in range(1, D_CONV):
                                    nc.vector.scalar_tensor_tensor(
                                        out=acc[j][:], in0=u0t[:, f * FC + j: f * FC + j + FC],
                                        scalar=convw[:, m8 * D_CONV + j: m8 * D_CONV + j + 1],
                                        in1=acc[j - 1][:], op0=ALU.mult, op1=ALU.add)
                                nc.scalar.activation(uc[m8][:, f * FC:(f + 1) * FC],
                                                     acc[D_CONV - 1][:], AF.Silu,
                                                     bias=convb[:, m8:m8 + 1])

            if STOP_AFTER == "A":
                stpool = dctx.enter_context(tc.tile_pool(name=f"st{p}", bufs=2))
                for mt in range(MT):
                    sc = stpool.tile([128, D_MODEL], F32, tag="sc")
                    nc.scalar.copy(sc[:], uc[mt % NB][:, 0:D_MODEL])
                    nc.sync.dma_start(io["out"][mt * 128:(mt + 1) * 128, :], sc[:])
                continue
            # ---------- phase B: xdbl = uc @ Wx ; dt = softplus(dtr @ Wdt + bdt) ----------
            dtpool = dctx.enter_context(tc.tile_pool(name=f"dtp{p}", bufs=1))
            dtt = [dtpool.tile([128, L], F16, tag=f"dt{d}", name=f"dt{d}") for d in range(NB)]
            with ExitStack() as bctx:
                bpool = bctx.enter_context(tc.tile_pool(name=f"pb{p}", bufs=1))
                psB = bctx.enter_context(tc.tile_pool(name=f"psB{p}", bufs=3, space="PSUM"))

                wdt = bpool.tile([32, 2 * D_INNER], BF16, tag="wdt")
                nc.sync.dma_start(wdt[:], io[f"{p}_Wdt"])
                bdt = bpool.tile([128, NB], F32, tag="bdt")
                nc.sync.dma_start(bdt[:], io[f"{p}_bdt"])
                dtr = bpool.tile([32, L], F32, tag="dtr")
                dtr_hi = bpool.tile([32, L], BF16, tag="dtr_hi")
                dtr_lo = bpool.tile([32, L], BF16, tag="dtr_lo")
                bmr = bpool.tile([16, L], BF16, tag="bmr")
                cmr = bpool.tile([16, L], BF16, tag="cmr")

                for f in range(FT):
                    ps = psB.tile([80, FC], F32, tag="pxd", bufs=2)
                    for k in range(NB):
                        nc.tensor.matmul(ps[:], wx[:, k * 80:(k + 1) * 80],
                                         uc[k][:, f * FC:(f + 1) * FC],
                                         start=(k == 0), stop=(k == NB - 1))
                    nc.scalar.copy(dtr[:, f * FC:(f + 1) * FC], ps[0:DT_RANK, :])
                    nc.scalar.copy(dtr_hi[:, f * FC:(f + 1) * FC], ps[0:DT_RANK, :])
                    # negated so that b = dtn*uc*(-Bm) = dt*u*Bm
                    nc.scalar.activation(bmr[:, f * FC:(f + 1) * FC],
                                         ps[DT_RANK:DT_RANK + D_STATE, :],
                                         AF.Copy, scale=-1.0)
                    nc.scalar.copy(cmr[:, f * FC:(f + 1) * FC], ps[64:80, :])
                nc.sync.dma_start(bc_d[0:16, :], bmr[:])
                nc.sync.dma_start(bc_d[16:32, :], cmr[:])
                # split-bf16 residual: dtr_lo = dtr - widen(dtr_hi)
                nc.vector.tensor_tensor(out=dtr_lo[:], in0=dtr[:], in1=dtr_hi[:],
                                        op=ALU.subtract)
                # dtn = -softplus(dtproj + bdt) = ln(sigmoid(-(dtproj + bdt)))
                # (walrus has no softplus table; bdt tile holds -bdt already)
                sg = [bpool.tile([128, L], F32, tag=f"sg{d}", name=f"sg{d}")
                      for d in range(NB)]
                for d in range(NB):
                    for f in range(FT):
                        ps = psB.tile([128, FC], F32, tag="pdt")
                        # split-bf16 product: Whi*hi + Whi*lo + Wlo*hi ~ fp32
                        nc.tensor.matmul(ps[:], wdt[:, d * 128:(d + 1) * 128],
                                         dtr_hi[:, f * FC:(f + 1) * FC],
                                         start=True, stop=False)
                        nc.tensor.matmul(ps[:], wdt[:, d * 128:(d + 1) * 128],
                                         dtr_lo[:, f * FC:(f + 1) * FC],
                                         start=False, stop=False)
                        nc.tensor.matmul(ps[:], wdt[:, D_INNER + d * 128: D_INNER + (d + 1) * 128],
                                         dtr_hi[:, f * FC:(f + 1) * FC],
                                         start=False, stop=True)
                        nc.scalar.activation(sg[d][:, f * FC:(f + 1) * FC], ps[:],
                                             AF.Sigmoid, bias=bdt[:, d:d + 1],
                                             scale=-1.0)
                for d in range(NB):
                    nc.scalar.activation(dtt[d][:], sg[d][:], AF.Ln)

            if STOP_AFTER == "B":
                stpool = dctx.enter_context(tc.tile_pool(name=f"st{p}", bufs=2))
                for mt in range(MT):
                    sc = stpool.tile([128, D_MODEL], F32, tag="sc")
                    nc.scalar.copy(sc[:], dtt[mt % NB][:, 0:D_MODEL])
                    nc.sync.dma_start(io["out"][mt * 128:(mt + 1) * 128, :], sc[:])
                continue
            # ---------- scan phase ----------
            yfpool = dctx.enter_context(tc.tile_pool(name=f"yfp{p}", bufs=1))
            yf = [yfpool.tile([128, L], BF16, tag=f"yf{d}", name=f"yf{d}") for d in range(NB)]
            with ExitStack() as sctx:
                scanpool = sctx.enter_context(tc.tile_pool(name=f"sc{p}", bufs=2))
                qpool = sctx.enter_context(tc.tile_pool(name=f"q{p}", bufs=3))
                bcpool = sctx.enter_context(tc.tile_pool(name=f"bc{p}", bufs=2))
                psY = sctx.enter_context(tc.tile_pool(name=f"psY{p}", bufs=2, space="PSUM"))

                for d in range(NB):
                    dtu = scanpool.tile([128, L], BF16, tag="dtu")
                    nc.vector.tensor_tensor(out=dtu[:], in0=dtt[d][:], in1=uc[d][:], op=ALU.mult)
                    zst = scanpool.tile([128, L], BF16, tag="zst")
                    nc.sync.dma_start(zst[:], zs_d[:, d * L:(d + 1) * L])
                    py = psY.tile([128, L], F32, tag="py")
                    for n in range(D_STATE):
                        a = scanpool.tile([128, L], F32, tag="a")
                        nc.scalar.activation(a[:], dtt[d][:], AF.Exp,
                                             scale=amat[:, d * D_STATE + n: d * D_STATE + n + 1])
                        bb = bcpool.tile([128, L], BF16, tag="bb")
                        nc.sync.dma_start(bb[:], bc_d[n:n + 1, :].broadcast_to((128, L)))
                        bt = scanpool.tile([128, L], bdtype, tag="bt")
                        nc.vector.tensor_tensor(out=bt[:], in0=dtu[:], in1=bb[:], op=ALU.mult)
                        h = scanpool.tile([128, L], BF16, tag="h")
                        nc.vector.tensor_tensor_scan(h[:], a[:], bt[:], 0.0, ALU.mult, ALU.add)
                        cb = bcpool.tile([128, L], BF16, tag="cb")
                        nc.sync.dma_start(cb[:], bc_d[16 + n:17 + n, :].broadcast_to((128, L)))
                        q = qpool.tile([128, L], BF16, tag="q")
                        nc.vector.tensor_tensor(out=q[:], in0=h[:], in1=cb[:], op=ALU.mult)
                        for f in range(FT):
                            nc.tensor.matmul(py[:, f * FC:(f + 1) * FC], ident[:],
                                             q[:, f * FC:(f + 1) * FC],
                                             start=(n == 0), stop=(n == D_STATE - 1))
                    yd = scanpool.tile([128, L], BF16, tag="yd")
                    nc.vector.scalar_tensor_tensor(out=yd[:], in0=uc[d][:],
                                                   scalar=dmat[:, d:d + 1], in1=py[:],
                                                   op0=ALU.mult, op1=ALU.add)
                    yf_dst = yf[d][:] if p == "f" else yf[d][:, ::-1]
                    nc.vector.tensor_tensor(out=yf_dst, in0=yd[:], in1=zst[:], op=ALU.mult)

            if STOP_AFTER == "S":
                stpool = dctx.enter_context(tc.tile_pool(name=f"st{p}", bufs=2))
                for mt in range(MT):
                    sc = stpool.tile([128, D_MODEL], F32, tag="sc")
                    nc.scalar.copy(sc[:], yf[mt % NB][:, 0:D_MODEL])
                    nc.sync.dma_start(io["out"][mt * 128:(mt + 1) * 128, :], sc[:])
                continue
            # ---------- output projection (token-major) + (bwd) layernorm ----------
            with ExitStack() as octx:
                psO = octx.enter_context(tc.tile_pool(name=f"psO{p}", bufs=4, space="PSUM"))
                lnpool = octx.enter_context(tc.tile_pool(name=f"ln{p}", bufs=2))
                for mt in range(MT):
                    po = psO.tile([128, D_MODEL], F32, tag="po")
                    for k in range(NB):
                        nc.tensor.matmul(po[:], yf[k][:, mt * 128:(mt + 1) * 128],
                                         wout[:, k * D_MODEL:(k + 1) * D_MODEL],
                                         start=(k == 0), stop=(k == NB - 1))
                    if p == "f":
                        st = lnpool.tile([128, D_MODEL], F32, tag="st")
                        nc.scalar.copy(st[:], po[:])
                        nc.sync.dma_start(s_d[:, mt * D_MODEL:(mt + 1) * D_MODEL], st[:])
                    else:
                        sf = lnpool.tile([128, D_MODEL], F32, tag="sf")
                        nc.sync.dma_start(sf[:], s_d[:, mt * D_MODEL:(mt + 1) * D_MODEL])
                        s = lnpool.tile([128, D_MODEL], F32, tag="s")
                        # tensor_tensor_reduce is broken on hw (NRT_EXEC_UNIT
                        # unrecoverable) — use TT add + tensor_reduce instead
                        nc.vector.tensor_tensor(out=s[:], in0=sf[:], in1=po[:], op=ALU.add)
                        ssum = lnpool.tile([128, 1], F32, tag="ssum")
                        nc.vector.tensor_reduce(ssum[:], s[:], axis=mybir.AxisListType.X,
                                                op=ALU.add)
                        nmu = lnpool.tile([128, 1], F32, tag="nmu")
                        nc.vector.tensor_scalar_mul(nmu[:], ssum[:], -1.0 / D_MODEL)
                        sq = lnpool.tile([128, D_MODEL], F32, tag="sq")
                        vsum = lnpool.tile([128, 1], F32, tag="vsum")
                        nc.scalar.activation(sq[:], s[:], AF.Square, bias=nmu[:],
                                             accum_out=vsum[:])
                        var = lnpool.tile([128, 1], F32, tag="var")
                        nc.vector.tensor_scalar(out=var[:], in0=vsum[:],
                                                scalar1=1.0 / D_MODEL, scalar2=LN_EPS,
                                                op0=ALU.mult, op1=ALU.add)
                        sd = lnpool.tile([128, 1], F32, tag="sd")
                        nc.scalar.activation(sd[:], var[:], AF.Sqrt)
                        rstd = lnpool.tile([128, 1], F32, tag="rstd")
                        nc.vector.reciprocal(rstd[:], sd[:])
                        xm = lnpool.tile([128, D_MODEL], F32, tag="xm")
                        nc.vector.tensor_scalar(out=xm[:], in0=s[:], scalar1=nmu[:],
                                                scalar2=rstd[:], op0=ALU.add, op1=ALU.mult)
                        o1 = lnpool.tile([128, D_MODEL], F32, tag="o1")
                        nc.vector.tensor_tensor(out=o1[:], in0=xm[:], in1=ln_g[:], op=ALU.mult)
                        o2 = lnpool.tile([128, D_MODEL], F32, tag="o2")
                        nc.vector.tensor_tensor(out=o2[:], in0=o1[:], in1=ln_b[:], op=ALU.add)
                        nc.sync.dma_start(io["out"][mt * 128:(mt + 1) * 128, :], o2[:])


def build_nc(L: int) -> tuple[bass.Bass, dict]:
    nc = bacc.Bacc("TRN2", target_bir_lowering=False, debug=False)
    io = declare_ios(nc, L)
    with tile.TileContext(nc) as tc:
        with ExitStack() as ctx:
            build_kernel(ctx, tc, io, L)
    nc.compile()
    return nc, io


# ----------------------------------------------------------------------------
# kernel entry point
# ----------------------------------------------------------------------------
_CACHE = {}


def _get_nc(L: int):
    if L not in _CACHE:
        _CACHE[L] = build_nc(L)
    return _CACHE[L]


def kernel(**inputs) -> np.ndarray:
    from concourse.bass_utils import run_bass_kernel_spmd

    in_maps, L = host_prep(inputs)
    nc, io = _get_nc(L)
    n = len(in_maps)
    res = run_bass_kernel_spmd(nc, in_maps, core_ids=list(range(n)))
    return np.stack([np.asarray(res.results[i]["out"], dtype=np.float32) for i in range(n)])


def kernel_timed(reps: int = 5, **inputs):
    """Run on hardware with device-resident inputs; returns (out, best_ns).

    best_ns is the minimum wall-clock of a full 8-core dispatch (includes
    PJRT/axon launch overhead, so it upper-bounds device exec time).
    """
    import time
    import jax
    from jax.sharding import Mesh, PartitionSpec
    from jax.experimental.shard_map import shard_map
    from concourse import bass2jax as b2j

    in_maps, L = host_prep(inputs)
    nc, io = _get_nc(L)
    n_cores = len(in_maps)
    b2j.install_neuronx_cc_hook()

    part_name = nc.partition_id_tensor.name if nc.partition_id_tensor else None
    in_names, out_names, out_avals, zero_outs = [], [], [], []
    for alloc in nc.m.functions[0].allocations:
        if not isinstance(alloc, mybir.MemoryLocationSet):
            continue
        name = alloc.memorylocations[0].name
        if alloc.kind == "ExternalInput":
            if name != part_name:
                in_names.append(name)
        elif alloc.kind == "ExternalOutput":
            out_names.append(name)
            shp = list(alloc.tensor_shape)
            npdt = mybir.dt.np(alloc.dtype)
            out_avals.append(jax.core.ShapedArray(shp, npdt))
            zero_outs.append(np.zeros(shp, npdt))
    n_params = len(in_names)
    n_outs = len(out_names)
    all_in_names = in_names + out_names
    if part_name is not None:
        all_in_names = all_in_names + [part_name]

    def _body(*args):
        operands = list(args)
        if part_name is not None:
            operands.append(b2j.partition_id_tensor())
        outs = b2j._bass_exec_p.bind(
            *operands, out_avals=tuple(out_avals), in_names=tuple(all_in_names),
            out_names=tuple(out_names), lowering_input_output_aliases=(),
            sim_require_finite=True, sim_require_nnan=True, nc=nc)
        return tuple(outs)

    devices = jax.devices()[:n_cores]
    mesh = Mesh(np.asarray(devices), ("core",))
    in_specs = (PartitionSpec("core"),) * (n_params + n_outs)
    out_specs = (PartitionSpec("core"),) * n_outs
    sharded = jax.jit(shard_map(_body, mesh=mesh, in_specs=in_specs,
                                out_specs=out_specs, check_rep=False),
                      keep_unused=True)
    concat_in = [np.concatenate([np.asarray(m[nm]) for m in in_maps], axis=0)
                 for nm in in_names]
    concat_zeros = [np.zeros((n_cores * z.shape[0], *z.shape[1:]), z.dtype)
                    for z in zero_outs]
    from jax.sharding import NamedSharding
    shard = NamedSharding(mesh, PartitionSpec("core"))
    dev_in = [jax.device_put(a, shard) for a in concat_in]
    dev_zero = [jax.device_put(a, shard) for a in concat_zeros]

    out_arrs = sharded(*dev_in, *dev_zero)           # warmup/compile
    jax.block_until_ready(out_arrs)
    # Per-dispatch wall time is dominated by the axon/PJRT tunnel (~75 ms for
    # a trivial kernel).  Estimate device exec time from the marginal cost of
    # pipelined async dispatches: (T(N2) - T(N1)) / (N2 - N1), which hides
    # the per-call launch overhead (a trivial kernel measures ~0.6 ms here).
    def timed(n):
        t0 = time.perf_counter()
        rs = [sharded(*dev_in, *dev_zero) for _ in range(n)]
        jax.block_until_ready(rs)
        return time.perf_counter() - t0
    n1, n2 = 16, 64
    t1s, t2s = [], []
    for _ in range(max(2, reps)):
        t1s.append(timed(n1))
        t2s.append(timed(n2))
    best = (min(t2s) - min(t1s)) / (n2 - n1)
    out = np.stack([
        np.asarray(out_arrs[0]).reshape(n_cores, *out_avals[0].shape)[c]
        for c in range(n_cores)
    ]).astype(np.float32)
    return out, best * 1e9


if __name__ == "__main__":
    import time
    npz = np.load("/tmp/inputs.npz")
    inputs = {k: npz[k] for k in npz.files}
    t0 = time.time()
    out = kernel(**inputs)
    print(f"kernel done in {time.time()-t0:.1f}s, out shape {out.shape}")
    out2, ns = kernel_timed(**inputs)
    print(f"timed: {ns:.0f} ns  ({ns/1e6:.3f} ms)")


# revision 23
# speedup vs baseline: 1.6014x; 1.6014x over previous
"""BiMamba block (bidirectional Mamba-1 + layernorm) as a Bass/Tile kernel
for 8 Trainium2 NeuronCores.

Sharding: data-parallel over batch — core i computes batch row i end-to-end
(both scan directions + layernorm), no collectives.

Per-core layout: channel-major [channel(partition), time(free)] activations
until the output projection, which emits token-major [token, d_model].

Selective scan per (d-block of 128 channels, state index n) on [128, L] tiles:
    a = exp(A[:,n] * dt)            ACT, per-partition scale AP
    b = (dt*u) * B_bcast[n]         DVE tensor_tensor bf16 (2x mode)
    h = tensor_tensor_scan(a, b)    DVE, fp32 internal recurrence state
    q = h * C_bcast[n]              DVE tensor_tensor bf16 (2x mode)
    psum_y += I @ q                 PE accumulates the sum over n in PSUM

B/C broadcasts are DMA re-reads of a small DRAM staging row with a
partition-step-0 access pattern.  The backward direction runs on
host-reversed input; un-reversal is free via a negative-stride output AP at
the yf write.
"""

import os
import sys
from contextlib import ExitStack

for _p in ("/opt/trn_rl_repo", "/root/.axon_site/_ro/trn_rl_repo"):
    if os.path.isdir(_p) and _p not in sys.path:
        sys.path.insert(0, _p)

import numpy as np
import ml_dtypes

import concourse.bass as bass
import concourse.tile as tile
from concourse import bacc, mybir
from concourse.masks import make_identity

AF = mybir.ActivationFunctionType
ALU = mybir.AluOpType
F32 = mybir.dt.float32
F32R = mybir.dt.float32r
BF16 = mybir.dt.bfloat16
F16 = mybir.dt.float16

D_MODEL = 512
D_STATE = 16
D_CONV = 4
D_INNER = 1024
DT_RANK = 32
NB = D_INNER // 128          # 8 d-blocks
KM = D_MODEL // 128          # 4 k-tiles over d_model
LN_EPS = 1e-5

XZ_F32R = False              # xz matmul in float32r (else bf16)
SCAN_B_BF16 = True           # scan data1 dtype bf16 (else fp32)
STOP_AFTER = os.environ.get("BIMAMBA_STOP_AFTER", "")


def host_prep(inputs: dict, l_override: int | None = None) -> tuple[list[dict], int]:
    """Full problem inputs -> per-core in_maps (one batch row per core)."""
    x = np.asarray(inputs["x"], dtype=np.float32)
    Bsz, L, _ = x.shape
    if l_override is not None:
        L = l_override
        x = x[:, :L]
    bf = ml_dtypes.bfloat16

    def pack(a, nblk):  # [nblk*128, F] -> [128, nblk*F]
        return np.concatenate([a[i * 128:(i + 1) * 128] for i in range(nblk)], axis=1).copy()

    shared = {}
    for p in ("f", "b"):
        Wxz = np.asarray(inputs[f"{p}_Wxz"], np.float32)
        shared[f"{p}_Wxz"] = pack(Wxz, KM) if XZ_F32R else pack(Wxz, KM).astype(bf)
        cw = np.asarray(inputs[f"{p}_conv_w"], np.float32).reshape(D_INNER, D_CONV)
        shared[f"{p}_convw"] = pack(cw, NB)
        cb = np.asarray(inputs[f"{p}_conv_b"], np.float32).reshape(D_INNER, 1)
        shared[f"{p}_convb"] = pack(cb, NB)
        Wxm = np.asarray(inputs[f"{p}_Wx"], np.float32)
        Wxp = np.zeros((D_INNER, 80), np.float32)
        Wxp[:, 0:48] = Wxm[:, 0:48]
        Wxp[:, 64:80] = Wxm[:, 48:64]
        shared[f"{p}_Wx"] = pack(Wxp, NB).astype(bf)
        Wdtf = np.asarray(inputs[f"{p}_Wdt"], np.float32)
        Wdt_hi = Wdtf.astype(bf)
        Wdt_lo = (Wdtf - Wdt_hi.astype(np.float32)).astype(bf)
        shared[f"{p}_Wdt"] = np.concatenate([Wdt_hi, Wdt_lo], axis=1).copy()
        nbdt = -np.asarray(inputs[f"{p}_bdt"], np.float32).reshape(D_INNER, 1)
        shared[f"{p}_bdt"] = pack(nbdt, NB)
        # negated-dt convention: dtn = -softplus(...) is stored, so the exp
        # scale must be -A = +exp(A_log)
        negA = np.exp(np.asarray(inputs[f"{p}_A_log"], np.float32))
        shared[f"{p}_A"] = pack(negA, NB)
        Dv = np.asarray(inputs[f"{p}_D"], np.float32).reshape(D_INNER, 1)
        shared[f"{p}_D"] = pack(Dv, NB)
        shared[f"{p}_Wout"] = pack(np.asarray(inputs[f"{p}_Wout"], np.float32), NB).astype(bf)
    shared["ln_g"] = np.broadcast_to(np.asarray(inputs["ln_g"], np.float32)[None, :], (128, D_MODEL)).copy()
    shared["ln_b"] = np.broadcast_to(np.asarray(inputs["ln_b"], np.float32)[None, :], (128, D_MODEL)).copy()

    in_maps = []
    for bi in range(Bsz):
        xT = np.ascontiguousarray(x[bi].T)        # [512, L]
        xTr = np.ascontiguousarray(x[bi][::-1].T)
        m = dict(shared)
        m["xT"] = pack(xT, KM) if XZ_F32R else pack(xT, KM).astype(bf)
        m["xTr"] = pack(xTr, KM) if XZ_F32R else pack(xTr, KM).astype(bf)
        in_maps.append(m)
    return in_maps, L


def declare_ios(nc: bass.Bass, L: int) -> dict:
    io = {}
    xdt = F32R if XZ_F32R else BF16
    io["xT"] = nc.dram_tensor("xT", [128, KM * L], xdt, kind="ExternalInput").ap()
    io["xTr"] = nc.dram_tensor("xTr", [128, KM * L], xdt, kind="ExternalInput").ap()
    for p in ("f", "b"):
        io[f"{p}_Wxz"] = nc.dram_tensor(f"{p}_Wxz", [128, KM * 2 * D_INNER], xdt, kind="ExternalInput").ap()
        io[f"{p}_convw"] = nc.dram_tensor(f"{p}_convw", [128, NB * D_CONV], F32, kind="ExternalInput").ap()
        io[f"{p}_convb"] = nc.dram_tensor(f"{p}_convb", [128, NB], F32, kind="ExternalInput").ap()
        io[f"{p}_Wx"] = nc.dram_tensor(f"{p}_Wx", [128, NB * 80], BF16, kind="ExternalInput").ap()
        io[f"{p}_Wdt"] = nc.dram_tensor(f"{p}_Wdt", [32, 2 * D_INNER], BF16, kind="ExternalInput").ap()
        io[f"{p}_bdt"] = nc.dram_tensor(f"{p}_bdt", [128, NB], F32, kind="ExternalInput").ap()
        io[f"{p}_A"] = nc.dram_tensor(f"{p}_A", [128, NB * D_STATE], F32, kind="ExternalInput").ap()
        io[f"{p}_D"] = nc.dram_tensor(f"{p}_D", [128, NB], F32, kind="ExternalInput").ap()
        io[f"{p}_Wout"] = nc.dram_tensor(f"{p}_Wout", [128, NB * D_MODEL], BF16, kind="ExternalInput").ap()
    io["ln_g"] = nc.dram_tensor("ln_g", [128, D_MODEL], F32, kind="ExternalInput").ap()
    io["ln_b"] = nc.dram_tensor("ln_b", [128, D_MODEL], F32, kind="ExternalInput").ap()
    io["out"] = nc.dram_tensor("out", [L, D_MODEL], F32, kind="ExternalOutput").ap()
    return io


def build_kernel(ctx: ExitStack, tc: tile.TileContext, io: dict, L: int):
    nc = tc.nc
    FC = min(512, L)
    FT = L // FC                 # 512-wide free chunks
    MT = L // 128                # token tiles
    xdt = F32R if XZ_F32R else BF16
    bdtype = BF16 if SCAN_B_BF16 else F32
    HM = D_INNER // 128          # m-tiles per xz half (8)

    wpool = ctx.enter_context(tc.tile_pool(name="wglob", bufs=1))
    ident = wpool.tile([128, 128], BF16, tag="ident")
    make_identity(nc, ident[:])
    ln_g = wpool.tile([128, D_MODEL], F32, tag="ln_g")
    nc.sync.dma_start(ln_g[:], io["ln_g"])
    ln_b = wpool.tile([128, D_MODEL], F32, tag="ln_b")
    nc.sync.dma_start(ln_b[:], io["ln_b"])
    dglob = ctx.enter_context(tc.tile_pool(name="dglob", bufs=1, space="DRAM"))
    s_d = dglob.tile([128, MT * D_MODEL], F32, tag="s_d")

    for p in ("f", "b"):
        with ExitStack() as dctx:
            awpool = dctx.enter_context(tc.tile_pool(name=f"aw{p}", bufs=1))
            ucpool = dctx.enter_context(tc.tile_pool(name=f"ucp{p}", bufs=1))
            dpool = dctx.enter_context(tc.tile_pool(name=f"dram{p}", bufs=1, space="DRAM"))
            zs_d = dpool.tile([128, NB * L], BF16, tag="zs_d")
            bc_d = dpool.tile([32, L], BF16, tag="bc_d")

            wx = awpool.tile([128, NB * 80], BF16, tag="wx")
            nc.sync.dma_start(wx[:], io[f"{p}_Wx"])
            amat = awpool.tile([128, NB * D_STATE], F32, tag="amat")
            nc.sync.dma_start(amat[:], io[f"{p}_A"])
            dmat = awpool.tile([128, NB], F32, tag="dmat")
            nc.sync.dma_start(dmat[:], io[f"{p}_D"])
            wout = awpool.tile([128, NB * D_MODEL], BF16, tag="wout")
            nc.sync.dma_start(wout[:], io[f"{p}_Wout"])

            uc = [ucpool.tile([128, L], BF16, tag=f"uc{d}", name=f"uc{d}") for d in range(NB)]

            # ---------- phase A: xz matmul (m-half-streamed weights), conv ----------
            with ExitStack() as actx:
                apool = actx.enter_context(tc.tile_pool(name=f"pa{p}", bufs=1))
                whpool = actx.enter_context(tc.tile_pool(name=f"wh{p}", bufs=2))
                u0pool = actx.enter_context(tc.tile_pool(name=f"u0p{p}", bufs=3))
                evpool = actx.enter_context(tc.tile_pool(name=f"ev{p}", bufs=3))
                convpool = actx.enter_context(tc.tile_pool(name=f"conv{p}", bufs=2))
                psA = actx.enter_context(tc.tile_pool(name=f"psA{p}", bufs=3, space="PSUM"))

                xin = apool.tile([128, KM * L], xdt, tag="xin")
                nc.sync.dma_start(xin[:], io["xT" if p == "f" else "xTr"])
                convw = apool.tile([128, NB * D_CONV], F32, tag="convw")
                nc.sync.dma_start(convw[:], io[f"{p}_convw"])
                convb = apool.tile([128, NB], F32, tag="convb")
                nc.sync.dma_start(convb[:], io[f"{p}_convb"])

                for half in range(2):        # 0: u-channels, 1: z-channels
                    wh = whpool.tile([128, KM * D_INNER], xdt, tag="wh")
                    wsrc = io[f"{p}_Wxz"].rearrange("p (k c) -> p k c", k=KM)[
                        :, :, half * D_INNER:(half + 1) * D_INNER]
                    nc.sync.dma_start(
                        wh[:].rearrange("p (k c) -> p k c", k=KM), wsrc)
                    for m8 in range(HM):
                        u0t = None
                        if half == 0:
                            u0t = u0pool.tile([128, D_CONV - 1 + L], BF16, tag="u0")
                            nc.gpsimd.memset(u0t[:, 0:D_CONV - 1], 0.0)
                        for f in range(FT):
                            ps = psA.tile([128, FC], F32, tag="pxz")
                            for k in range(KM):
                                nc.tensor.matmul(
                                    ps[:],
                                    wh[:, k * D_INNER + m8 * 128: k * D_INNER + (m8 + 1) * 128],
                                    xin[:, k * L + f * FC: k * L + (f + 1) * FC],
                                    start=(k == 0), stop=(k == KM - 1),
                                )
                            if half == 0:
                                nc.scalar.copy(
                                    u0t[:, D_CONV - 1 + f * FC: D_CONV - 1 + (f + 1) * FC], ps[:])
                            else:
                                zt = evpool.tile([128, FC], BF16, tag="zt")
                                nc.scalar.activation(zt[:], ps[:], AF.Silu)
                                nc.sync.dma_start(
                                    zs_d[:, m8 * L + f * FC: m8 * L + (f + 1) * FC], zt[:])
                        if half == 0:
                            # depthwise causal conv + silu for d-block m8
                            for f in range(FT):
                                acc = [convpool.tile([128, FC], F32, tag=f"cacc{j % 2}",
                                                     name=f"cacc{j}") for j in range(D_CONV)]
                                nc.vector.tensor_scalar_mul(
                                    acc[0][:], u0t[:, f * FC: f * FC + FC],
                                    convw[:, m8 * D_CONV: m8 * D_CONV + 1])
                                for j in range(1, D_CONV):
                                    nc.vector.scalar_tensor_tensor(
                                        out=acc[j][:], in0=u0t[:, f * FC + j: f * FC + j + FC],
                                        scalar=convw[:, m8 * D_CONV + j: m8 * D_CONV + j + 1],
                                        in1=acc[j - 1][:], op0=ALU.mult, op1=ALU.add)
                                nc.scalar.activation(uc[m8][:, f * FC:(f + 1) * FC],
                                                     acc[D_CONV - 1][:], AF.Silu,
                                                     bias=convb[:, m8:m8 + 1])

            if STOP_AFTER == "A":
                stpool = dctx.enter_context(tc.tile_pool(name=f"st{p}", bufs=2))
                for mt in range(MT):
                    sc = stpool.tile([128, D_MODEL], F32, tag="sc")
                    nc.scalar.copy(sc[:], uc[mt % NB][:, 0:D_MODEL])
                    nc.sync.dma_start(io["out"][mt * 128:(mt + 1) * 128, :], sc[:])
                continue
            # ---------- phase B: xdbl = uc @ Wx ; dt = softplus(dtr @ Wdt + bdt) ----------
            dtpool = dctx.enter_context(tc.tile_pool(name=f"dtp{p}", bufs=1))
            dtt = [dtpool.tile([128, L], F16, tag=f"dt{d}", name=f"dt{d}") for d in range(NB)]
            with ExitStack() as bctx:
                bpool = bctx.enter_context(tc.tile_pool(name=f"pb{p}", bufs=1))
                psB = bctx.enter_context(tc.tile_pool(name=f"psB{p}", bufs=3, space="PSUM"))

                wdt = bpool.tile([32, 2 * D_INNER], BF16, tag="wdt")
                nc.sync.dma_start(wdt[:], io[f"{p}_Wdt"])
                bdt = bpool.tile([128, NB], F32, tag="bdt")
                nc.sync.dma_start(bdt[:], io[f"{p}_bdt"])
                dtr = bpool.tile([32, L], F32, tag="dtr")
                dtr_hi = bpool.tile([32, L], BF16, tag="dtr_hi")
                dtr_lo = bpool.tile([32, L], BF16, tag="dtr_lo")
                bmr = bpool.tile([16, L], BF16, tag="bmr")
                cmr = bpool.tile([16, L], BF16, tag="cmr")

                for f in range(FT):
                    ps = psB.tile([80, FC], F32, tag="pxd", bufs=2)
                    for k in range(NB):
                        nc.tensor.matmul(ps[:], wx[:, k * 80:(k + 1) * 80],
                                         uc[k][:, f * FC:(f + 1) * FC],
                                         start=(k == 0), stop=(k == NB - 1))
                    nc.scalar.copy(dtr[:, f * FC:(f + 1) * FC], ps[0:DT_RANK, :])
                    nc.scalar.copy(dtr_hi[:, f * FC:(f + 1) * FC], ps[0:DT_RANK, :])
                    # negated so that b = dtn*uc*(-Bm) = dt*u*Bm
                    nc.scalar.activation(bmr[:, f * FC:(f + 1) * FC],
                                         ps[DT_RANK:DT_RANK + D_STATE, :],
                                         AF.Copy, scale=-1.0)
                    nc.scalar.copy(cmr[:, f * FC:(f + 1) * FC], ps[64:80, :])
                nc.sync.dma_start(bc_d[0:16, :], bmr[:])
                nc.sync.dma_start(bc_d[16:32, :], cmr[:])
                # split-bf16 residual: dtr_lo = dtr - widen(dtr_hi)
                nc.vector.tensor_tensor(out=dtr_lo[:], in0=dtr[:], in1=dtr_hi[:],
                                        op=ALU.subtract)
                # dtn = -softplus(dtproj + bdt) = ln(sigmoid(-(dtproj + bdt)))
                # (walrus has no softplus table; bdt tile holds -bdt already)
                sg = [bpool.tile([128, L], F32, tag=f"sg{d}", name=f"sg{d}")
                      for d in range(NB)]
                for d in range(NB):
                    for f in range(FT):
                        ps = psB.tile([128, FC], F32, tag="pdt")
                        # split-bf16 product: Whi*hi + Whi*lo + Wlo*hi ~ fp32
                        nc.tensor.matmul(ps[:], wdt[:, d * 128:(d + 1) * 128],
                                         dtr_hi[:, f * FC:(f + 1) * FC],
                                         start=True, stop=False)
                        nc.tensor.matmul(ps[:], wdt[:, d * 128:(d + 1) * 128],
                                         dtr_lo[:, f * FC:(f + 1) * FC],
                                         start=False, stop=False)
                        nc.tensor.matmul(ps[:], wdt[:, D_INNER + d * 128: D_INNER + (d + 1) * 128],
                                         dtr_hi[:, f * FC:(f + 1) * FC],
                                         start=False, stop=True)
                        nc.scalar.activation(sg[d][:, f * FC:(f + 1) * FC], ps[:],
                                             AF.Sigmoid, bias=bdt[:, d:d + 1],
                                             scale=-1.0)
                for d in range(NB):
                    nc.scalar.activation(dtt[d][:], sg[d][:], AF.Ln)

            if STOP_AFTER == "B":
                stpool = dctx.enter_context(tc.tile_pool(name=f"st{p}", bufs=2))
                for mt in range(MT):
                    sc = stpool.tile([128, D_MODEL], F32, tag="sc")
                    nc.scalar.copy(sc[:], dtt[mt % NB][:, 0:D_MODEL])
                    nc.sync.dma_start(io["out"][mt * 128:(mt + 1) * 128, :], sc[:])
                continue
            # ---------- scan phase ----------
            yfpool = dctx.enter_context(tc.tile_pool(name=f"yfp{p}", bufs=1))
            yf = [yfpool.tile([128, L], BF16, tag=f"yf{d}", name=f"yf{d}") for d in range(NB)]
            with ExitStack() as sctx:
                scanpool = sctx.enter_context(tc.tile_pool(name=f"sc{p}", bufs=2))
                qpool = sctx.enter_context(tc.tile_pool(name=f"q{p}", bufs=3))
                bcpool = sctx.enter_context(tc.tile_pool(name=f"bc{p}", bufs=2))
                psY = sctx.enter_context(tc.tile_pool(name=f"psY{p}", bufs=2, space="PSUM"))

                for d in range(NB):
                    dtu = scanpool.tile([128, L], BF16, tag="dtu", bufs=1)
                    nc.vector.tensor_tensor(out=dtu[:], in0=dtt[d][:], in1=uc[d][:], op=ALU.mult)
                    zst = scanpool.tile([128, L], BF16, tag="zst", bufs=1)
                    nc.sync.dma_start(zst[:], zs_d[:, d * L:(d + 1) * L])
                    py = psY.tile([128, L], F32, tag="py")
                    for np2 in range(D_STATE // 2):
                        n0 = 2 * np2
                        # pair adjacent n: one C-multiply + one q tile per pair
                        cb2 = bcpool.tile([128, 2, L], BF16, tag="cb")
                        nc.sync.dma_start(cb2[:, 0, :], bc_d[16 + n0:17 + n0, :].broadcast_to((128, L)))
                        nc.sync.dma_start(cb2[:, 1, :], bc_d[17 + n0:18 + n0, :].broadcast_to((128, L)))
                        h2 = scanpool.tile([128, 2, L], BF16, tag="h")
                        for i in (0, 1):
                            n = n0 + i
                            a = scanpool.tile([128, L], F32, tag="a")
                            nc.scalar.activation(a[:], dtt[d][:], AF.Exp,
                                                 scale=amat[:, d * D_STATE + n: d * D_STATE + n + 1])
                            bb = bcpool.tile([128, L], BF16, tag="bb")
                            nc.sync.dma_start(bb[:], bc_d[n:n + 1, :].broadcast_to((128, L)))
                            bt = scanpool.tile([128, L], bdtype, tag="bt")
                            nc.vector.tensor_tensor(out=bt[:], in0=dtu[:], in1=bb[:], op=ALU.mult)
                            nc.vector.tensor_tensor_scan(h2[:, i, :], a[:], bt[:], 0.0,
                                                         ALU.mult, ALU.add)
                        q2 = qpool.tile([128, 2, L], BF16, tag="q", bufs=2)
                        nc.vector.tensor_tensor(out=q2[:], in0=h2[:], in1=cb2[:], op=ALU.mult)
                        for i in (0, 1):
                            for f in range(FT):
                                nc.tensor.matmul(py[:, f * FC:(f + 1) * FC], ident[:],
                                                 q2[:, i, f * FC:(f + 1) * FC],
                                                 start=(n0 + i == 0), stop=(n0 + i == D_STATE - 1))
                    yd = scanpool.tile([128, L], BF16, tag="yd")
                    nc.vector.scalar_tensor_tensor(out=yd[:], in0=uc[d][:],
                                                   scalar=dmat[:, d:d + 1], in1=py[:],
                                                   op0=ALU.mult, op1=ALU.add)
                    yf_dst = yf[d][:] if p == "f" else yf[d][:, ::-1]
                    nc.vector.tensor_tensor(out=yf_dst, in0=yd[:], in1=zst[:], op=ALU.mult)

            if STOP_AFTER == "S":
                stpool = dctx.enter_context(tc.tile_pool(name=f"st{p}", bufs=2))
                for mt in range(MT):
                    sc = stpool.tile([128, D_MODEL], F32, tag="sc")
                    nc.scalar.copy(sc[:], yf[mt % NB][:, 0:D_MODEL])
                    nc.sync.dma_start(io["out"][mt * 128:(mt + 1) * 128, :], sc[:])
                continue
            # ---------- output projection (token-major) + (bwd) layernorm ----------
            with ExitStack() as octx:
                psO = octx.enter_context(tc.tile_pool(name=f"psO{p}", bufs=4, space="PSUM"))
                lnpool = octx.enter_context(tc.tile_pool(name=f"ln{p}", bufs=2))
                for mt in range(MT):
                    po = psO.tile([128, D_MODEL], F32, tag="po")
                    for k in range(NB):
                        nc.tensor.matmul(po[:], yf[k][:, mt * 128:(mt + 1) * 128],
                                         wout[:, k * D_MODEL:(k + 1) * D_MODEL],
                                         start=(k == 0), stop=(k == NB - 1))
                    if p == "f":
                        st = lnpool.tile([128, D_MODEL], F32, tag="st")
                        nc.scalar.copy(st[:], po[:])
                        nc.sync.dma_start(s_d[:, mt * D_MODEL:(mt + 1) * D_MODEL], st[:])
                    else:
                        sf = lnpool.tile([128, D_MODEL], F32, tag="sf")
                        nc.sync.dma_start(sf[:], s_d[:, mt * D_MODEL:(mt + 1) * D_MODEL])
                        s = lnpool.tile([128, D_MODEL], F32, tag="s")
                        # tensor_tensor_reduce is broken on hw (NRT_EXEC_UNIT
                        # unrecoverable) — use TT add + tensor_reduce instead
                        nc.vector.tensor_tensor(out=s[:], in0=sf[:], in1=po[:], op=ALU.add)
                        ssum = lnpool.tile([128, 1], F32, tag="ssum")
                        nc.vector.tensor_reduce(ssum[:], s[:], axis=mybir.AxisListType.X,
                                                op=ALU.add)
                        nmu = lnpool.tile([128, 1], F32, tag="nmu")
                        nc.vector.tensor_scalar_mul(nmu[:], ssum[:], -1.0 / D_MODEL)
                        sq = lnpool.tile([128, D_MODEL], F32, tag="sq")
                        vsum = lnpool.tile([128, 1], F32, tag="vsum")
                        nc.scalar.activation(sq[:], s[:], AF.Square, bias=nmu[:],
                                             accum_out=vsum[:])
                        var = lnpool.tile([128, 1], F32, tag="var")
                        nc.vector.tensor_scalar(out=var[:], in0=vsum[:],
                                                scalar1=1.0 / D_MODEL, scalar2=LN_EPS,
                                                op0=ALU.mult, op1=ALU.add)
                        sd = lnpool.tile([128, 1], F32, tag="sd")
                        nc.scalar.activation(sd[:], var[:], AF.Sqrt)
                        rstd = lnpool.tile([128, 1], F32, tag="rstd")
                        nc.vector.reciprocal(rstd[:], sd[:])
                        xm = lnpool.tile([128, D_MODEL], F32, tag="xm")
                        nc.vector.tensor_scalar(out=xm[:], in0=s[:], scalar1=nmu[:],
                                                scalar2=rstd[:], op0=ALU.add, op1=ALU.mult)
                        o1 = lnpool.tile([128, D_MODEL], F32, tag="o1")
                        nc.vector.tensor_tensor(out=o1[:], in0=xm[:], in1=ln_g[:], op=ALU.mult)
                        o2 = lnpool.tile([128, D_MODEL], F32, tag="o2")
                        nc.vector.tensor_tensor(out=o2[:], in0=o1[:], in1=ln_b[:], op=ALU.add)
                        nc.sync.dma_start(io["out"][mt * 128:(mt + 1) * 128, :], o2[:])


def build_nc(L: int) -> tuple[bass.Bass, dict]:
    nc = bacc.Bacc("TRN2", target_bir_lowering=False, debug=False)
    io = declare_ios(nc, L)
    with tile.TileContext(nc) as tc:
        with ExitStack() as ctx:
            build_kernel(ctx, tc, io, L)
    nc.compile()
    return nc, io


# ----------------------------------------------------------------------------
# kernel entry point
# ----------------------------------------------------------------------------
_CACHE = {}


def _get_nc(L: int):
    if L not in _CACHE:
        _CACHE[L] = build_nc(L)
    return _CACHE[L]


def kernel(**inputs) -> np.ndarray:
    from concourse.bass_utils import run_bass_kernel_spmd

    in_maps, L = host_prep(inputs)
    nc, io = _get_nc(L)
    n = len(in_maps)
    res = run_bass_kernel_spmd(nc, in_maps, core_ids=list(range(n)))
    return np.stack([np.asarray(res.results[i]["out"], dtype=np.float32) for i in range(n)])


def kernel_timed(reps: int = 5, **inputs):
    """Run on hardware with device-resident inputs; returns (out, best_ns).

    best_ns is the minimum wall-clock of a full 8-core dispatch (includes
    PJRT/axon launch overhead, so it upper-bounds device exec time).
    """
    import time
    import jax
    from jax.sharding import Mesh, PartitionSpec
    from jax.experimental.shard_map import shard_map
    from concourse import bass2jax as b2j

    in_maps, L = host_prep(inputs)
    nc, io = _get_nc(L)
    n_cores = len(in_maps)
    b2j.install_neuronx_cc_hook()

    part_name = nc.partition_id_tensor.name if nc.partition_id_tensor else None
    in_names, out_names, out_avals, zero_outs = [], [], [], []
    for alloc in nc.m.functions[0].allocations:
        if not isinstance(alloc, mybir.MemoryLocationSet):
            continue
        name = alloc.memorylocations[0].name
        if alloc.kind == "ExternalInput":
            if name != part_name:
                in_names.append(name)
        elif alloc.kind == "ExternalOutput":
            out_names.append(name)
            shp = list(alloc.tensor_shape)
            npdt = mybir.dt.np(alloc.dtype)
            out_avals.append(jax.core.ShapedArray(shp, npdt))
            zero_outs.append(np.zeros(shp, npdt))
    n_params = len(in_names)
    n_outs = len(out_names)
    all_in_names = in_names + out_names
    if part_name is not None:
        all_in_names = all_in_names + [part_name]

    def _body(*args):
        operands = list(args)
        if part_name is not None:
            operands.append(b2j.partition_id_tensor())
        outs = b2j._bass_exec_p.bind(
            *operands, out_avals=tuple(out_avals), in_names=tuple(all_in_names),
            out_names=tuple(out_names), lowering_input_output_aliases=(),
            sim_require_finite=True, sim_require_nnan=True, nc=nc)
        return tuple(outs)

    devices = jax.devices()[:n_cores]
    mesh = Mesh(np.asarray(devices), ("core",))
    in_specs = (PartitionSpec("core"),) * (n_params + n_outs)
    out_specs = (PartitionSpec("core"),) * n_outs
    sharded = jax.jit(shard_map(_body, mesh=mesh, in_specs=in_specs,
                                out_specs=out_specs, check_rep=False),
                      keep_unused=True)
    concat_in = [np.concatenate([np.asarray(m[nm]) for m in in_maps], axis=0)
                 for nm in in_names]
    concat_zeros = [np.zeros((n_cores * z.shape[0], *z.shape[1:]), z.dtype)
                    for z in zero_outs]
    from jax.sharding import NamedSharding
    shard = NamedSharding(mesh, PartitionSpec("core"))
    dev_in = [jax.device_put(a, shard) for a in concat_in]
    dev_zero = [jax.device_put(a, shard) for a in concat_zeros]

    out_arrs = sharded(*dev_in, *dev_zero)           # warmup/compile
    jax.block_until_ready(out_arrs)
    # Per-dispatch wall time is dominated by the axon/PJRT tunnel (~75 ms for
    # a trivial kernel).  Estimate device exec time from the marginal cost of
    # pipelined async dispatches: (T(N2) - T(N1)) / (N2 - N1), which hides
    # the per-call launch overhead (a trivial kernel measures ~0.6 ms here).
    def timed(n):
        t0 = time.perf_counter()
        rs = [sharded(*dev_in, *dev_zero) for _ in range(n)]
        jax.block_until_ready(rs)
        return time.perf_counter() - t0
    n1, n2 = 16, 64
    t1s, t2s = [], []
    for _ in range(max(2, reps)):
        t1s.append(timed(n1))
        t2s.append(timed(n2))
    best = (min(t2s) - min(t1s)) / (n2 - n1)
    out = np.stack([
        np.asarray(out_arrs[0]).reshape(n_cores, *out_avals[0].shape)[c]
        for c in range(n_cores)
    ]).astype(np.float32)
    return out, best * 1e9


if __name__ == "__main__":
    import time
    npz = np.load("/tmp/inputs.npz")
    inputs = {k: npz[k] for k in npz.files}
    t0 = time.time()
    out = kernel(**inputs)
    print(f"kernel done in {time.time()-t0:.1f}s, out shape {out.shape}")
    out2, ns = kernel_timed(**inputs)
    print(f"timed: {ns:.0f} ns  ({ns/1e6:.3f} ms)")
